# revision 21
# baseline (speedup 1.0000x reference)
"""MAM dense kernel for Trainium2 (8 NeuronCores, SPMD data-parallel over M).

C[m,n] = max_k(x[m,k]*w[n,k]) + min_k(x[m,k]*w[n,k]) + bias[n]

Strategy per core (M_c = 512 rows of x):
  - Layout: n on partitions (8 tiles of 128 n's), k on the free axis.
  - For each group of J m-rows: broadcast those rows across all 128
    partitions via a stride-0 DMA from DRAM, then on the Vector engine:
      q = w * x_bcast            (tensor_tensor mult)
      max tree: log2(K) rounds of pairwise tensor_tensor max over halves
      min tree: same with min
    fp16 tiles run the tree rounds in the DVE 2x_1P perf mode.
  - Combine max+min+bias in fp32, store transposed output [N, M_c];
    the host transposes back and concatenates core results.

PRECISION:
  'a' — cast x,w to fp16; fp16 products (fastest, rel err ~2e-3)
  'b' — fp32 inputs, products rounded to fp16 (rel err ~1e-3)
  'c' — all fp32 (bit-exact vs fp32 reference, slowest)
"""

import os
import sys

sys.path.insert(0, "/opt/trn_rl_repo")

import numpy as np

M, K, N = 4096, 1024, 1024
N_CORES = 8
M_C = M // N_CORES  # 512 rows per core
NT = N // 128  # 8 n-tiles

PRECISION = "a"

_last_results = None  # BassKernelResults from the most recent run (for test.py)


def _build_nc(n_groups=None, nt=NT, j=None, k=K, precision=None):
    import concourse.bacc as bacc
    import concourse.mybir as mybir
    import concourse.tile as tile
    from contextlib import ExitStack

    precision = precision or PRECISION
    # fp32 tiles are twice the size; halve the group to fit SBUF
    if j is None:
        j = 2 if precision == "c" else 4
    if n_groups is None:
        n_groups = M_C // j

    f32 = mybir.dt.float32
    f16 = mybir.dt.float16
    mult = mybir.AluOpType.mult
    amax = mybir.AluOpType.max
    amin = mybir.AluOpType.min
    aadd = mybir.AluOpType.add

    in_dt = f16 if precision == "a" else f32  # dtype of w/x operand tiles
    q_dt = f32 if precision == "c" else f16  # dtype of products + trees
    in_sz = 2 if precision == "a" else 4

    m_c = n_groups * j
    n_total = nt * 128

    nc = bacc.Bacc("TRN2", target_bir_lowering=False, debug=False)
    x_d = nc.dram_tensor("x", [m_c, k], f32, kind="ExternalInput").ap()
    w_d = nc.dram_tensor("w", [n_total, k], f32, kind="ExternalInput").ap()
    b_d = nc.dram_tensor("b", [n_total], f32, kind="ExternalInput").ap()
    o_d = nc.dram_tensor("o", [n_total, m_c], f32, kind="ExternalOutput").ap()
    # broadcast-source copy of x in the operand dtype
    xs_d = nc.dram_tensor("xsd", [m_c, k], in_dt).ap()

    with tile.TileContext(nc) as tc, ExitStack() as ctx:
        p_const = ctx.enter_context(tc.tile_pool(name="const", bufs=1))

        # --- preamble: load w (+ cast), stage x into xs_d (broadcast source).
        # No slot reuse here: the direct2d DMA encoding supports one wait.
        w_sb = p_const.tile([128, nt, k], in_dt)
        b_sb = p_const.tile([128, nt], f32)
        out_sb = p_const.tile([128, nt, m_c], f32)
        with tc.tile_pool(name="stage", bufs=1) as p_stage:
            # x roundtrip first: the first broadcast DMA depends on it
            if in_dt is f16:
                x32 = p_stage.tile([128, j, k], f32)
                x16t = p_stage.tile([128, j, k], f16)
                nc.sync.dma_start(
                    x32[:n_groups], x_d.rearrange("(p jj) k -> p jj k", jj=j)
                )
                nc.vector.tensor_copy(x16t[:n_groups], x32[:n_groups])
                nc.sync.dma_start(
                    xs_d.rearrange("(p jj) k -> p jj k", jj=j), x16t[:n_groups]
                )

                w32 = p_stage.tile([128, nt, k], f32)
                nc.sync.dma_start(w32[:], w_d.rearrange("(t p) k -> p t k", p=128))
                nc.vector.tensor_copy(w_sb[:], w32[:])
            else:
                # straight fp32 copy of x to the broadcast scratch (chunks
                # of <=128 partition-rows; n_groups can exceed 128)
                x_v = x_d.rearrange("(p jj) k -> p jj k", jj=j)
                xs_v = xs_d.rearrange("(p jj) k -> p jj k", jj=j)
                for base in range(0, n_groups, 128):
                    c = min(128, n_groups - base)
                    xcp = p_stage.tile([128, j, k], f32, tag=f"xcp{base}")
                    nc.sync.dma_start(xcp[:c], x_v[base : base + c])
                    nc.sync.dma_start(xs_v[base : base + c], xcp[:c])
                nc.sync.dma_start(w_sb[:], w_d.rearrange("(t p) k -> p t k", p=128))

            nc.sync.dma_start(b_sb[:], b_d.rearrange("(t p) -> p t", p=128))

        p_xb = ctx.enter_context(
            tc.tile_pool(name="xb", bufs=3 if precision == "a" else 2)
        )
        p_q = ctx.enter_context(tc.tile_pool(name="q", bufs=1))
        p_a = ctx.enter_context(tc.tile_pool(name="ta", bufs=1))
        p_b = ctx.enter_context(tc.tile_pool(name="tb", bufs=1))
        p_r = ctx.enter_context(tc.tile_pool(name="r", bufs=2))

        w_b = w_sb[:].unsqueeze(2).broadcast_to([128, nt, j, k])

        for g in range(n_groups):
            # broadcast this group's j rows of x to all partitions (from DRAM)
            xb = p_xb.tile([128, j, k], in_dt)
            src = (
                xs_d[g * j : (g + 1) * j, :]
                .rearrange("j k -> (j k)")
                .unsqueeze(0)
                .broadcast_to([128, j * k])
            )
            nc.sync.dma_start(xb[:].rearrange("p j k -> p (j k)"), src)

            # products: q[p_n, t, jj, k] = w[p_n, t, k] * x[g*j+jj, k]
            q = p_q.tile([128, nt, j, k], q_dt)
            xb_b = xb[:].unsqueeze(1).broadcast_to([128, nt, j, k])
            nc.vector.tensor_tensor(q[:], w_b, xb_b, mult)

            # pairwise-halves reduction trees, tensor_reduce tail at f=16
            ta = p_a.tile([128, nt, j, k // 2], q_dt)
            tb = p_b.tile([128, nt, j, k // 4], q_dt)
            results = {}
            for op_name, op in (("mx", amax), ("mn", amin)):
                res = p_r.tile([128, nt, j], f32, tag=op_name)
                cur = q[:]
                f = k // 2
                use_a = True
                while f >= 16:
                    dst = (ta if use_a else tb)[:, :, :, 0:f]
                    nc.vector.tensor_tensor(
                        dst, cur[:, :, :, 0:f], cur[:, :, :, f : 2 * f], op
                    )
                    cur = dst
                    use_a = not use_a
                    f //= 2
                nc.vector.tensor_reduce(
                    res[:], cur[:, :, :, 0 : 2 * f], axis=mybir.AxisListType.X, op=op
                )
                results[op_name] = res

            # combine: out[n, m] = max + min (bias folded in at the end)
            nc.vector.tensor_tensor(
                out_sb[:, :, g * j : (g + 1) * j],
                results["mx"][:],
                results["mn"][:],
                aadd,
            )
            # halfway through, add bias to + store the finished half so the
            # output DMA overlaps the second half's compute
            if g + 1 == n_groups // 2:
                half = (n_groups // 2) * j
                bias_h = b_sb[:].unsqueeze(2).broadcast_to([128, nt, half])
                nc.vector.tensor_tensor(
                    out_sb[:, :, :half], out_sb[:, :, :half], bias_h, aadd
                )
                nc.sync.dma_start(
                    o_d.rearrange("(t p) m -> p t m", p=128)[:, :, :half],
                    out_sb[:, :, :half],
                )

        # --- bias + store for the second half
        half = (n_groups // 2) * j
        bias_h = b_sb[:].unsqueeze(2).broadcast_to([128, nt, m_c - half])
        nc.vector.tensor_tensor(
            out_sb[:, :, half:], out_sb[:, :, half:], bias_h, aadd
        )
        nc.sync.dma_start(
            o_d.rearrange("(t p) m -> p t m", p=128)[:, :, half:],
            out_sb[:, :, half:],
        )

    nc.compile()
    return nc


def kernel(x: np.ndarray, weight: np.ndarray, bias: np.ndarray) -> np.ndarray:
    global _last_results
    from concourse.bass_utils import run_bass_kernel_spmd

    try:  # NTFF tracing needs antenv.axon_hooks; disable if unavailable
        import antenv.axon_hooks  # noqa: F401
    except ImportError:
        os.environ["BASS_NEVER_TRACE"] = "1"

    x = np.ascontiguousarray(x, dtype=np.float32)
    weight = np.ascontiguousarray(weight, dtype=np.float32)
    bias = np.ascontiguousarray(bias, dtype=np.float32)

    nc = _build_nc()
    core_ids = list(range(N_CORES))
    in_maps = [
        {"x": x[c * M_C : (c + 1) * M_C], "w": weight, "b": bias} for c in core_ids
    ]
    res = run_bass_kernel_spmd(nc, in_maps, core_ids)
    _last_results = res

    out = np.empty((M, N), dtype=np.float32)
    for c in core_ids:
        out[c * M_C : (c + 1) * M_C, :] = res.results[c]["o"].T
    return out
